# revision 2
# baseline (speedup 1.0000x reference)
"""Trainium2 Bass kernel for a 2-layer relational GraphSAGE VGAE encoder.

Contract: kernel(**inputs) takes the FULL unsharded inputs (as produced by
setup_inputs()) and returns the full (mu, logvar) tuple.

Strategy (8 NeuronCores, SPMD single NEFF):
  - Nodes block-sharded: core c owns nodes [c*2500, (c+1)*2500), padded to 2560.
  - Edges partitioned by destination-node owner; segment-mean is computed
    locally as a sequence of (gather 128 src rows) @ (host-built one-hot with
    1/cnt weights) matmuls that directly emit the feature-major result.
  - Everything on device is feature-major [channels, nodes] fp16 with fp32
    PSUM accumulation. BatchNorm (eval) is folded into the layer-2 weights on
    the host; layer-2 has a single stacked projection weight [2560 -> 1024]
    covering mu_l|lv_l (aggregated side) and mu_r|lv_r (self side).
  - The aggregated-side projections are PE-transposed to node-major and
    AllGather'd (fp16) so each core can gather h-projections of any source
    node; the self-side matmuls overlap the collective.
"""
import sys

sys.path.insert(0, "/opt/trn_rl_repo")

import numpy as np

NCORES = 8
N = 20000
E = 100000
IN = 512
HID = 512
CAT = 2560
OUT = 256
BN_EPS = 1e-5

NLOC = N // NCORES          # 2500
NPAD = 2560                 # 20 * 128, 5 * 512
NG = NPAD // 512            # 5 node groups of 512 per core
NREL = 5
P = 128
TABROWS = NCORES * NPAD     # 20480


# ----------------------------------------------------------------------------
# Host-side preprocessing: sharding, edge chunking, weight folding
# ----------------------------------------------------------------------------

def _chunk_edges(key, ncells, src_vals, col, val):
    """Group edges by per-core cell, chunk each cell into 128-edge chunks.

    key: [E] int = core * ncells + cell   (cell < ncells)
    src_vals: [E] int32 gather row index for each edge
    col: [E] int in [0, 512) one-hot column (dst position within node group)
    val: [E] f32 one-hot value (1/cnt)

    Returns: nch [ncells] shared chunk counts (max over cores, >=1),
             base [ncells] chunk base offsets, Ctot,
             idxT [NCORES, 128, Ctot] int32, vals [NCORES, Ctot, 128, 512] f16
    """
    counts = np.bincount(key, minlength=NCORES * ncells).reshape(NCORES, ncells)
    nch = np.maximum((counts + P - 1) // P, 1).max(axis=0)  # [ncells]
    base = np.concatenate([[0], np.cumsum(nch)[:-1]])
    Ctot = int(nch.sum())

    order = np.argsort(key, kind="stable")
    ks = key[order]
    # position of each sorted edge within its (core, cell) run
    first_of_run = np.r_[True, ks[1:] != ks[:-1]]
    run_starts = np.flatnonzero(first_of_run)
    run_id = np.cumsum(first_of_run) - 1
    pos = np.arange(len(ks)) - run_starts[run_id]

    core_s = ks // ncells
    cell_s = ks % ncells
    chunk_s = base[cell_s] + pos // P
    row_s = pos % P

    idxT = np.zeros((NCORES, P, Ctot), np.int32)
    vals = np.zeros((NCORES, Ctot, P, 512), np.float16)
    idxT[core_s, row_s, chunk_s] = src_vals[order]
    vals[core_s, chunk_s, row_s, col[order]] = val[order]
    return nch, base, Ctot, idxT, vals


def _preprocess(x, edge_index, edge_attr, Wl5, Wr5, bl5,
                Wmu_l, Wmu_r, bmu, Wlv_l, Wlv_r, blv,
                gamma, beta, run_mean, run_var):
    x = np.asarray(x, np.float32)
    src = np.asarray(edge_index[0], np.int64)
    dst = np.asarray(edge_index[1], np.int64)
    rel = np.asarray(edge_attr, np.int64)

    # --- per-node degree counts ---
    cnt1 = np.bincount(rel * N + dst, minlength=NREL * N).reshape(NREL, N)
    inv1 = 1.0 / np.maximum(cnt1, 1.0)
    cnt2 = np.bincount(dst, minlength=N)
    inv2 = 1.0 / np.maximum(cnt2, 1.0)

    core = dst // NLOC
    loc = dst % NLOC
    g = loc // 512
    col = loc % 512

    # layer-1 cells: (rel, group); gather rows straight from x table
    key1 = (core * NREL + rel) * NG + g
    nch1, base1, C1, a1i, a1v = _chunk_edges(
        key1, NREL * NG, src.astype(np.int32), col, inv1[rel, dst])

    # layer-2 cells: (group); gather rows from the all-gathered table
    tabrow = ((src // NLOC) * NPAD + (src % NLOC)).astype(np.int32)
    key2 = core * NG + g
    nch2, base2, C2, a2i, a2v = _chunk_edges(key2, NG, tabrow, col, inv2[dst])

    # --- node features ---
    xtab = x.astype(np.float16)                           # [N, 512] gather table
    xt = np.zeros((NCORES, IN, NPAD), np.float16)         # feature-major local x
    for c in range(NCORES):
        xt[c, :, :NLOC] = x[c * NLOC:(c + 1) * NLOC].T

    # --- weight folding (BN eval folded into layer-2 weights) ---
    f64 = np.float64
    s = np.asarray(gamma, f64) / np.sqrt(np.asarray(run_var, f64) + BN_EPS)
    t = np.asarray(beta, f64) - np.asarray(run_mean, f64) * s

    wlt = np.ascontiguousarray(
        np.asarray(Wl5, np.float32).transpose(0, 2, 1)).astype(np.float16)
    wrt = np.ascontiguousarray(
        np.asarray(Wr5, np.float32).transpose(0, 2, 1)).astype(np.float16)

    Wtab = np.concatenate([np.asarray(Wmu_l, f64), np.asarray(Wlv_l, f64)], 0)
    Wself = np.concatenate([np.asarray(Wmu_r, f64), np.asarray(Wlv_r, f64)], 0)
    Wall = np.concatenate([Wtab * s[None, :], Wself * s[None, :]], 0)  # [1024, 2560]
    wallt = np.ascontiguousarray(Wall.T).astype(np.float16)            # [2560, 1024]

    tW = (Wtab @ t).astype(np.float32)                                  # [512]
    bself = (Wself @ t + np.concatenate(
        [np.asarray(bmu, f64), np.asarray(blv, f64)])).astype(np.float32)

    # bias tiles, laid out [128, n] so a column is a per-partition scalar
    blb = np.ascontiguousarray(
        np.asarray(bl5, np.float32).reshape(NREL * 4, P).T)   # [128, 20]
    twb = np.ascontiguousarray(tW.reshape(4, P).T)            # [128, 4]
    bsb = np.ascontiguousarray(bself.reshape(4, P).T)         # [128, 4]

    meta = (tuple(nch1), tuple(base1), C1, tuple(nch2), tuple(base2), C2)
    in_maps = []
    for c in range(NCORES):
        in_maps.append({
            "xtab": xtab, "xt": xt[c],
            "a1i": a1i[c], "a1v": a1v[c],
            "a2i": a2i[c], "a2v": a2v[c],
            "wlt": wlt, "wrt": wrt, "wallt": wallt,
            "blb": blb, "twb": twb, "bsb": bsb,
        })
    return meta, in_maps


# ----------------------------------------------------------------------------
# Device kernel
# ----------------------------------------------------------------------------

def _build(meta):
    import concourse.bacc as bacc
    import concourse.bass as bass
    import concourse.tile as tile
    import concourse.mybir as mybir
    from concourse.masks import make_identity

    nch1, base1, C1, nch2, base2, C2 = meta
    nch1 = np.asarray(nch1).reshape(NREL, NG)
    base1 = np.asarray(base1).reshape(NREL, NG)
    nch2 = np.asarray(nch2)
    base2 = np.asarray(base2)

    f16, f32, i32 = mybir.dt.float16, mybir.dt.float32, mybir.dt.int32

    nc = bacc.Bacc("TRN2", target_bir_lowering=False, debug=False,
                   num_devices=NCORES)

    xtab_t = nc.dram_tensor("xtab", [N, IN], f16, kind="ExternalInput")
    xt_t = nc.dram_tensor("xt", [IN, NPAD], f16, kind="ExternalInput")
    a1i_t = nc.dram_tensor("a1i", [P, C1], i32, kind="ExternalInput")
    a1v_t = nc.dram_tensor("a1v", [C1, P, 512], f16, kind="ExternalInput")
    a2i_t = nc.dram_tensor("a2i", [P, C2], i32, kind="ExternalInput")
    a2v_t = nc.dram_tensor("a2v", [C2, P, 512], f16, kind="ExternalInput")
    wlt_t = nc.dram_tensor("wlt", [NREL, IN, HID], f16, kind="ExternalInput")
    wrt_t = nc.dram_tensor("wrt", [NREL, IN, HID], f16, kind="ExternalInput")
    wallt_t = nc.dram_tensor("wallt", [CAT, 1024], f16, kind="ExternalInput")
    blb_t = nc.dram_tensor("blb", [P, NREL * 4], f32, kind="ExternalInput")
    twb_t = nc.dram_tensor("twb", [P, 4], f32, kind="ExternalInput")
    bsb_t = nc.dram_tensor("bsb", [P, 4], f32, kind="ExternalInput")
    out_t = nc.dram_tensor("out", [512, NPAD], f32, kind="ExternalOutput")

    hrelu = nc.dram_tensor("hrelu", [CAT, NPAD], f16, kind="Internal")
    ag_in = nc.dram_tensor("ag_in", [NPAD, 512], f16, kind="Internal")
    ag_tab = nc.dram_tensor("ag_tab", [TABROWS, 512], f16,
                            kind="Internal", addr_space="Shared")

    with tile.TileContext(nc) as tc:
        with (
            tc.tile_pool(name="constp", bufs=1) as constp,
            tc.tile_pool(name="resp", bufs=1) as resp,
            tc.tile_pool(name="wp", bufs=2) as wp,
            tc.tile_pool(name="iop", bufs=3) as iop,
            tc.tile_pool(name="actp", bufs=2) as actp,
            tc.tile_pool(name="psum", bufs=2, space="PSUM") as pp,
        ):
            # ---- constants / resident tiles ----
            ident = constp.tile([P, P], f16, name="ident", tag="ident")
            make_identity(nc, ident[:])
            blb_sb = constp.tile([P, NREL * 4], f32, name="blb_sb", tag="blb")
            nc.sync.dma_start(out=blb_sb[:], in_=blb_t.ap())
            twb_sb = constp.tile([P, 4], f32, name="twb_sb", tag="twb")
            nc.sync.dma_start(out=twb_sb[:], in_=twb_t.ap())
            bsb_sb = constp.tile([P, 4], f32, name="bsb_sb", tag="bsb")
            nc.sync.dma_start(out=bsb_sb[:], in_=bsb_t.ap())

            idx1_sb = resp.tile([P, C1], i32, name="idx1_sb", tag="idx1")
            nc.sync.dma_start(out=idx1_sb[:], in_=a1i_t.ap())
            idx2_sb = resp.tile([P, C2], i32, name="idx2_sb", tag="idx2")
            nc.sync.dma_start(out=idx2_sb[:], in_=a2i_t.ap())

            # local x, feature-major, resident: 20 tiles [128, 512]
            xt_sb = []
            for gg in range(NG):
                row = []
                for kc in range(4):
                    tl = resp.tile([P, 512], f16, name=f"xt_{gg}_{kc}",
                                   tag=f"xt_{gg}_{kc}")
                    nc.sync.dma_start(
                        out=tl[:],
                        in_=xt_t.ap()[kc * P:(kc + 1) * P,
                                      gg * 512:(gg + 1) * 512])
                    row.append(tl)
                xt_sb.append(row)

            # stacked projection weightT, resident: 20 tiles [128, 1024]
            wall_sb = []
            for r in range(20):
                tl = resp.tile([P, 1024], f16, name=f"wall_{r}", tag=f"wall_{r}")
                nc.sync.dma_start(out=tl[:], in_=wallt_t.ap()[r * P:(r + 1) * P, :])
                wall_sb.append(tl)

            # self-side projections, kept resident until the final combine
            self_res = [[None] * 4 for _ in range(NG)]

            # =========== Phase 1: layer-1 relational SAGE -> hrelu ==========
            for k in range(NREL):
                wl_sb, wr_sb = [], []
                for kc in range(4):
                    wl = wp.tile([P, HID], f16, name=f"wl_{k}_{kc}", tag=f"wl{kc}")
                    nc.sync.dma_start(
                        out=wl[:], in_=wlt_t.ap()[k, kc * P:(kc + 1) * P, :])
                    wl_sb.append(wl)
                    wr = wp.tile([P, HID], f16, name=f"wr_{k}_{kc}", tag=f"wr{kc}")
                    nc.sync.dma_start(
                        out=wr[:], in_=wrt_t.ap()[k, kc * P:(kc + 1) * P, :])
                    wr_sb.append(wr)

                for gg in range(NG):
                    nchunks = int(nch1[k, gg])
                    cbase = int(base1[k, gg])
                    # --- aggregation: mean_k^T for this node group ---
                    mean_ps = pp.tile([P, 2048], f32, space="PSUM",
                                      name=f"agg_{k}_{gg}", tag="big")
                    for ci in range(nchunks):
                        j = cbase + ci
                        gth = iop.tile([P, 512], f16, name=f"g1_{k}_{gg}_{ci}",
                                       tag="gth")
                        nc.gpsimd.indirect_dma_start(
                            out=gth[:], out_offset=None,
                            in_=xtab_t.ap(),
                            in_offset=bass.IndirectOffsetOnAxis(
                                ap=idx1_sb[:, j:j + 1], axis=0))
                        av = iop.tile([P, 512], f16, name=f"a1_{k}_{gg}_{ci}",
                                      tag="av")
                        nc.sync.dma_start(out=av[:], in_=a1v_t.ap()[j])
                        for cc in range(4):
                            nc.tensor.matmul(
                                out=mean_ps[:, cc * 512:(cc + 1) * 512],
                                lhsT=gth[:, cc * P:(cc + 1) * P],
                                rhs=av[:],
                                start=(ci == 0), stop=(ci == nchunks - 1))
                    mean_sb = []
                    for cc in range(4):
                        m = actp.tile([P, 512], f16, name=f"mean_{k}_{gg}_{cc}",
                                      tag=f"mean{cc}")
                        nc.vector.tensor_copy(
                            out=m[:], in_=mean_ps[:, cc * 512:(cc + 1) * 512])
                        mean_sb.append(m)

                    # --- dense: h = relu(Wl@mean + Wr@x + b) ---
                    h_ps = pp.tile([P, 2048], f32, space="PSUM",
                                   name=f"h_{k}_{gg}", tag="big")
                    for mc in range(4):
                        o = h_ps[:, mc * 512:(mc + 1) * 512]
                        for kc in range(4):
                            nc.tensor.matmul(
                                out=o, lhsT=wl_sb[kc][:, mc * P:(mc + 1) * P],
                                rhs=mean_sb[kc][:], start=(kc == 0), stop=False)
                        for kc in range(4):
                            nc.tensor.matmul(
                                out=o, lhsT=wr_sb[kc][:, mc * P:(mc + 1) * P],
                                rhs=xt_sb[gg][kc][:], start=False, stop=(kc == 3))
                    for mc in range(4):
                        r = actp.tile([P, 512], f16, name=f"relu_{k}_{gg}_{mc}",
                                      tag=f"relu{mc}")
                        nc.vector.tensor_scalar(
                            out=r[:], in0=h_ps[:, mc * 512:(mc + 1) * 512],
                            scalar1=blb_sb[:, k * 4 + mc:k * 4 + mc + 1],
                            scalar2=0.0,
                            op0=mybir.AluOpType.add, op1=mybir.AluOpType.max)
                        nc.sync.dma_start(
                            out=hrelu.ap()[k * 512 + mc * P:k * 512 + (mc + 1) * P,
                                           gg * 512:(gg + 1) * 512],
                            in_=r[:])

            # ====== Phase 2a: aggregated-side projections -> transpose -> ag_in
            for gg in range(NG):
                hr_sb = []
                for r in range(20):
                    tl = actp.tile([P, 512], f16, name=f"hrA_{gg}_{r}",
                                   tag=f"hr{r}", bufs=1)
                    nc.sync.dma_start(
                        out=tl[:],
                        in_=hrelu.ap()[r * P:(r + 1) * P, gg * 512:(gg + 1) * 512])
                    hr_sb.append(tl)
                agin_sb = [actp.tile([P, 512], f16, name=f"agin_{gg}_{ns}",
                                     tag=f"agin{ns}") for ns in range(4)]
                for mc in range(4):
                    p_ps = pp.tile([P, 2048], f32, space="PSUM",
                                   name=f"proj_{gg}_{mc}", tag="big")
                    o = p_ps[:, 0:512]
                    for r in range(20):
                        nc.tensor.matmul(
                            out=o, lhsT=wall_sb[r][:, mc * P:(mc + 1) * P],
                            rhs=hr_sb[r][:], start=(r == 0), stop=(r == 19))
                    tab = actp.tile([P, 512], f16, name=f"tab_{gg}_{mc}",
                                    tag=f"tab{mc}")
                    nc.vector.tensor_scalar(
                        out=tab[:], in0=o, scalar1=twb_sb[:, mc:mc + 1],
                        scalar2=None, op0=mybir.AluOpType.add)
                    for ns in range(4):
                        tr_ps = pp.tile([P, 2048], f16, space="PSUM",
                                        name=f"tr_{gg}_{mc}_{ns}", tag="big")
                        nc.tensor.transpose(
                            out=tr_ps[:, 0:P],
                            in_=tab[:, ns * P:(ns + 1) * P],
                            identity=ident[:])
                        nc.vector.tensor_copy(
                            out=agin_sb[ns][:, mc * P:(mc + 1) * P],
                            in_=tr_ps[:, 0:P])
                for ns in range(4):
                    nc.sync.dma_start(
                        out=ag_in.ap()[gg * 512 + ns * P:gg * 512 + (ns + 1) * P, :],
                        in_=agin_sb[ns][:])

            # =========== Phase 3: AllGather the projected table ==============
            nc.gpsimd.collective_compute(
                "AllGather", mybir.AluOpType.bypass,
                replica_groups=[list(range(NCORES))],
                ins=[ag_in.ap()], outs=[ag_tab.ap()])

            # ====== Phase 2b: self-side projections (overlap the collective)
            for gg in range(NG):
                hr_sb = []
                for r in range(20):
                    tl = actp.tile([P, 512], f16, name=f"hrB_{gg}_{r}",
                                   tag=f"hr{r}", bufs=1)
                    nc.sync.dma_start(
                        out=tl[:],
                        in_=hrelu.ap()[r * P:(r + 1) * P, gg * 512:(gg + 1) * 512])
                    hr_sb.append(tl)
                for mc in range(4):
                    p_ps = pp.tile([P, 2048], f32, space="PSUM",
                                   name=f"self_{gg}_{mc}", tag="big")
                    o = p_ps[:, 0:512]
                    for r in range(20):
                        nc.tensor.matmul(
                            out=o, lhsT=wall_sb[r][:, (4 + mc) * P:(5 + mc) * P],
                            rhs=hr_sb[r][:], start=(r == 0), stop=(r == 19))
                    sf = resp.tile([P, 512], f32, name=f"selfr_{gg}_{mc}",
                                   tag=f"self_{gg}_{mc}")
                    nc.vector.tensor_scalar(
                        out=sf[:], in0=o, scalar1=bsb_sb[:, mc:mc + 1],
                        scalar2=None, op0=mybir.AluOpType.add)
                    self_res[gg][mc] = sf

            # =========== Phase 4: layer-2 aggregation + combine ==============
            for gg in range(NG):
                nchunks = int(nch2[gg])
                cbase = int(base2[gg])
                m2_ps = pp.tile([P, 2048], f32, space="PSUM",
                                name=f"m2_{gg}", tag="big")
                for ci in range(nchunks):
                    j = cbase + ci
                    gth = iop.tile([P, 512], f16, name=f"g2_{gg}_{ci}", tag="gth")
                    nc.gpsimd.indirect_dma_start(
                        out=gth[:], out_offset=None,
                        in_=ag_tab.ap(),
                        in_offset=bass.IndirectOffsetOnAxis(
                            ap=idx2_sb[:, j:j + 1], axis=0))
                    av = iop.tile([P, 512], f16, name=f"a2_{gg}_{ci}", tag="av")
                    nc.sync.dma_start(out=av[:], in_=a2v_t.ap()[j])
                    for cc in range(4):
                        nc.tensor.matmul(
                            out=m2_ps[:, cc * 512:(cc + 1) * 512],
                            lhsT=gth[:, cc * P:(cc + 1) * P],
                            rhs=av[:],
                            start=(ci == 0), stop=(ci == nchunks - 1))
                for mc in range(4):
                    ob = actp.tile([P, 512], f32, name=f"out_{gg}_{mc}", tag="outsb")
                    nc.vector.tensor_tensor(
                        out=ob[:], in0=m2_ps[:, mc * 512:(mc + 1) * 512],
                        in1=self_res[gg][mc][:], op=mybir.AluOpType.add)
                    nc.sync.dma_start(
                        out=out_t.ap()[mc * P:(mc + 1) * P,
                                       gg * 512:(gg + 1) * 512],
                        in_=ob[:])

    nc.compile()
    return nc


# ----------------------------------------------------------------------------
# Entry point
# ----------------------------------------------------------------------------

_CACHE = {}


def build_and_run(inputs, trace=False, trace_kwargs=None):
    from concourse import bass_utils

    meta, in_maps = _preprocess(**inputs)
    if meta not in _CACHE:
        _CACHE[meta] = _build(meta)
    nc = _CACHE[meta]
    res = bass_utils.run_bass_kernel_spmd(
        nc, in_maps, core_ids=list(range(NCORES)),
        trace=trace, **(trace_kwargs or {}))

    mu = np.empty((N, OUT), np.float32)
    lv = np.empty((N, OUT), np.float32)
    for c in range(NCORES):
        blk = res.results[c]["out"]            # [512, 2560] fp32
        mu[c * NLOC:(c + 1) * NLOC] = blk[0:OUT, :NLOC].T
        lv[c * NLOC:(c + 1) * NLOC] = blk[OUT:2 * OUT, :NLOC].T
    return (mu, lv), res


def kernel(**inputs):
    out, _ = build_and_run(inputs, trace=False)
    return out


# revision 7
# speedup vs baseline: 1.2754x; 1.2754x over previous
"""Trainium2 Bass kernel for a 2-layer relational GraphSAGE VGAE encoder.

Contract: kernel(**inputs) takes the FULL unsharded inputs (as produced by
setup_inputs()) and returns the full (mu, logvar) tuple.

Strategy (8 NeuronCores, SPMD single NEFF):
  - Nodes block-sharded: core c owns nodes [c*2500, (c+1)*2500), padded to 2560.
  - Edges partitioned by destination-node owner; segment-mean is computed
    locally as a sequence of (gather 128 src rows) @ (host-built one-hot with
    1/cnt weights) matmuls that directly emit the feature-major result.
  - Everything on device is feature-major [channels, nodes] fp16 with fp32
    PSUM accumulation. BatchNorm (eval) is folded into the layer-2 weights on
    the host; layer-2 has a single stacked projection weight [2560 -> 1024]
    covering mu_l|lv_l (aggregated side) and mu_r|lv_r (self side).
  - The aggregated-side projections are PE-transposed to node-major and
    AllGather'd (fp16) so each core can gather h-projections of any source
    node; the self-side matmuls overlap the collective.
"""
import sys

sys.path.insert(0, "/opt/trn_rl_repo")

import numpy as np

NCORES = 8
N = 20000
E = 100000
IN = 512
HID = 512
CAT = 2560
OUT = 256
BN_EPS = 1e-5

NLOC = N // NCORES          # 2500
NPAD = 2560                 # 20 * 128, 5 * 512
NG = NPAD // 512            # 5 node groups of 512 per core
NREL = 5
P = 128
TABROWS = NCORES * NPAD     # 20480


# ----------------------------------------------------------------------------
# Host-side preprocessing: sharding, edge chunking, weight folding
# ----------------------------------------------------------------------------

def _chunk_edges(key, ncells, src_vals, col, val):
    """Group edges by per-core cell, chunk each cell into 128-edge chunks.

    key: [E] int = core * ncells + cell   (cell < ncells)
    src_vals: [E] int32 gather row index for each edge
    col: [E] int in [0, 512) one-hot column (dst position within node group)
    val: [E] f32 one-hot value (1/cnt)

    Returns: nch [ncells] shared chunk counts (max over cores, >=1),
             base [ncells] chunk base offsets, Ctot,
             idxT [NCORES, 128, Ctot] int32, vals [NCORES, Ctot, 128, 512] f16
    """
    counts = np.bincount(key, minlength=NCORES * ncells).reshape(NCORES, ncells)
    nch = np.maximum((counts + P - 1) // P, 1).max(axis=0)  # [ncells]
    base = np.concatenate([[0], np.cumsum(nch)[:-1]])
    Ctot = int(nch.sum())

    order = np.argsort(key, kind="stable")
    ks = key[order]
    # position of each sorted edge within its (core, cell) run
    first_of_run = np.r_[True, ks[1:] != ks[:-1]]
    run_starts = np.flatnonzero(first_of_run)
    run_id = np.cumsum(first_of_run) - 1
    pos = np.arange(len(ks)) - run_starts[run_id]

    core_s = ks // ncells
    cell_s = ks % ncells
    chunk_s = base[cell_s] + pos // P
    row_s = pos % P

    idxT = np.zeros((NCORES, P, Ctot), np.int32)
    vals = np.zeros((NCORES, Ctot, P, 512), np.float16)
    idxT[core_s, row_s, chunk_s] = src_vals[order]
    vals[core_s, chunk_s, row_s, col[order]] = val[order]
    return nch, base, Ctot, idxT, vals


def _preprocess(x, edge_index, edge_attr, Wl5, Wr5, bl5,
                Wmu_l, Wmu_r, bmu, Wlv_l, Wlv_r, blv,
                gamma, beta, run_mean, run_var):
    x = np.asarray(x, np.float32)
    src = np.asarray(edge_index[0], np.int64)
    dst = np.asarray(edge_index[1], np.int64)
    rel = np.asarray(edge_attr, np.int64)

    # --- per-node degree counts ---
    cnt1 = np.bincount(rel * N + dst, minlength=NREL * N).reshape(NREL, N)
    inv1 = 1.0 / np.maximum(cnt1, 1.0)
    cnt2 = np.bincount(dst, minlength=N)
    inv2 = 1.0 / np.maximum(cnt2, 1.0)

    core = dst // NLOC
    loc = dst % NLOC
    g = loc // 512
    col = loc % 512

    # layer-1 cells: (rel, group); gather rows straight from x table
    key1 = (core * NREL + rel) * NG + g
    nch1, base1, C1, a1i, a1v = _chunk_edges(
        key1, NREL * NG, src.astype(np.int32), col, inv1[rel, dst])

    # layer-2 cells: (group); gather rows from the all-gathered table
    tabrow = ((src // NLOC) * NPAD + (src % NLOC)).astype(np.int32)
    key2 = core * NG + g
    nch2, base2, C2, a2i, a2v = _chunk_edges(key2, NG, tabrow, col, inv2[dst])

    # --- node features ---
    xtab = x.astype(np.float16)                           # [N, 512] gather table
    xt = np.zeros((NCORES, IN, NPAD), np.float16)         # feature-major local x
    for c in range(NCORES):
        xt[c, :, :NLOC] = x[c * NLOC:(c + 1) * NLOC].T

    # --- weight folding (BN eval folded into layer-2 weights) ---
    f64 = np.float64
    s = np.asarray(gamma, f64) / np.sqrt(np.asarray(run_var, f64) + BN_EPS)
    t = np.asarray(beta, f64) - np.asarray(run_mean, f64) * s

    wlt = np.ascontiguousarray(
        np.asarray(Wl5, np.float32).transpose(0, 2, 1)).astype(np.float16)
    wrt = np.ascontiguousarray(
        np.asarray(Wr5, np.float32).transpose(0, 2, 1)).astype(np.float16)

    Wtab = np.concatenate([np.asarray(Wmu_l, f64), np.asarray(Wlv_l, f64)], 0)
    Wself = np.concatenate([np.asarray(Wmu_r, f64), np.asarray(Wlv_r, f64)], 0)
    Wall = np.concatenate([Wtab * s[None, :], Wself * s[None, :]], 0)  # [1024, 2560]
    wallt = np.ascontiguousarray(Wall.T).astype(np.float16)            # [2560, 1024]

    tW = (Wtab @ t).astype(np.float32)                                  # [512]
    bself = (Wself @ t + np.concatenate(
        [np.asarray(bmu, f64), np.asarray(blv, f64)])).astype(np.float32)

    # bias tiles, laid out [128, n] so a column is a per-partition scalar
    blb = np.ascontiguousarray(
        np.asarray(bl5, np.float32).reshape(NREL * 4, P).T)   # [128, 20]
    twb = np.ascontiguousarray(tW.reshape(4, P).T)            # [128, 4]
    bsb = np.ascontiguousarray(bself.reshape(4, P).T)         # [128, 4]

    meta = (tuple(nch1), tuple(base1), C1, tuple(nch2), tuple(base2), C2)
    in_maps = []
    for c in range(NCORES):
        in_maps.append({
            "xtab": xtab, "xt": xt[c],
            "a1i": a1i[c], "a1v": a1v[c],
            "a2i": a2i[c], "a2v": a2v[c],
            "wlt": wlt, "wrt": wrt, "wallt": wallt,
            "blb": blb, "twb": twb, "bsb": bsb,
        })
    return meta, in_maps


# ----------------------------------------------------------------------------
# Device kernel
# ----------------------------------------------------------------------------

def _build(meta):
    import concourse.bacc as bacc
    import concourse.bass as bass
    import concourse.tile as tile
    import concourse.mybir as mybir
    from concourse.masks import make_identity

    nch1, base1, C1, nch2, base2, C2 = meta
    nch1 = np.asarray(nch1).reshape(NREL, NG)
    base1 = np.asarray(base1).reshape(NREL, NG)
    nch2 = np.asarray(nch2)
    base2 = np.asarray(base2)

    f16, f32, i32 = mybir.dt.float16, mybir.dt.float32, mybir.dt.int32

    nc = bacc.Bacc("TRN2", target_bir_lowering=False, debug=False,
                   num_devices=NCORES)

    xtab_t = nc.dram_tensor("xtab", [N, IN], f16, kind="ExternalInput")
    xt_t = nc.dram_tensor("xt", [IN, NPAD], f16, kind="ExternalInput")
    a1i_t = nc.dram_tensor("a1i", [P, C1], i32, kind="ExternalInput")
    a1v_t = nc.dram_tensor("a1v", [C1, P, 512], f16, kind="ExternalInput")
    a2i_t = nc.dram_tensor("a2i", [P, C2], i32, kind="ExternalInput")
    a2v_t = nc.dram_tensor("a2v", [C2, P, 512], f16, kind="ExternalInput")
    wlt_t = nc.dram_tensor("wlt", [NREL, IN, HID], f16, kind="ExternalInput")
    wrt_t = nc.dram_tensor("wrt", [NREL, IN, HID], f16, kind="ExternalInput")
    wallt_t = nc.dram_tensor("wallt", [CAT, 1024], f16, kind="ExternalInput")
    blb_t = nc.dram_tensor("blb", [P, NREL * 4], f32, kind="ExternalInput")
    twb_t = nc.dram_tensor("twb", [P, 4], f32, kind="ExternalInput")
    bsb_t = nc.dram_tensor("bsb", [P, 4], f32, kind="ExternalInput")
    out_t = nc.dram_tensor("out", [512, NPAD], f32, kind="ExternalOutput")

    hrelu = nc.dram_tensor("hrelu", [CAT, NPAD], f16, kind="Internal")
    ag_in = nc.dram_tensor("ag_in", [NPAD, 512], f16, kind="Internal")
    ag_tab = nc.dram_tensor("ag_tab", [TABROWS, 512], f16,
                            kind="Internal", addr_space="Shared")

    with tile.TileContext(nc) as tc:
        with (
            tc.tile_pool(name="constp", bufs=1) as constp,
            tc.tile_pool(name="resp", bufs=1) as resp,
            tc.tile_pool(name="wp", bufs=2) as wp,
            tc.tile_pool(name="iop", bufs=3) as iop,
            tc.tile_pool(name="actp", bufs=2) as actp,
            tc.tile_pool(name="psum", bufs=2, space="PSUM") as pp,
        ):
            # ---- constants / resident tiles ----
            # (bulk resident preloads go on the scalar HWDGE ring so the sync
            # ring serves the steady-state A-value/weight stream immediately)
            ident = constp.tile([P, P], f16, name="ident", tag="ident")
            make_identity(nc, ident[:])
            blb_sb = constp.tile([P, NREL * 4], f32, name="blb_sb", tag="blb")
            nc.scalar.dma_start(out=blb_sb[:], in_=blb_t.ap())
            twb_sb = constp.tile([P, 4], f32, name="twb_sb", tag="twb")
            nc.scalar.dma_start(out=twb_sb[:], in_=twb_t.ap())
            bsb_sb = constp.tile([P, 4], f32, name="bsb_sb", tag="bsb")
            nc.scalar.dma_start(out=bsb_sb[:], in_=bsb_t.ap())

            idx1_sb = resp.tile([P, C1], i32, name="idx1_sb", tag="idx1")
            nc.sync.dma_start(out=idx1_sb[:], in_=a1i_t.ap())
            idx2_sb = resp.tile([P, C2], i32, name="idx2_sb", tag="idx2")
            nc.sync.dma_start(out=idx2_sb[:], in_=a2i_t.ap())

            # local x, feature-major, resident: 20 tiles [128, 512]
            xt_sb = []
            for gg in range(NG):
                row = []
                for kc in range(4):
                    tl = resp.tile([P, 512], f16, name=f"xt_{gg}_{kc}",
                                   tag=f"xt_{gg}_{kc}")
                    nc.scalar.dma_start(
                        out=tl[:],
                        in_=xt_t.ap()[kc * P:(kc + 1) * P,
                                      gg * 512:(gg + 1) * 512])
                    row.append(tl)
                xt_sb.append(row)

            # stacked projection weightT, resident: 20 tiles [128, 1024]
            wall_sb = []
            for r in range(20):
                tl = resp.tile([P, 1024], f16, name=f"wall_{r}", tag=f"wall_{r}")
                nc.scalar.dma_start(out=tl[:], in_=wallt_t.ap()[r * P:(r + 1) * P, :])
                wall_sb.append(tl)

            # self-side projections, kept resident until the final combine
            self_res = [[None] * 4 for _ in range(NG)]

            # =========== Phase 1: layer-1 relational SAGE -> hrelu ==========
            for k in range(NREL):
                wl_sb, wr_sb = [], []
                for kc in range(4):
                    wl = wp.tile([P, HID], f16, name=f"wl_{k}_{kc}", tag=f"wl{kc}")
                    nc.sync.dma_start(
                        out=wl[:], in_=wlt_t.ap()[k, kc * P:(kc + 1) * P, :])
                    wl_sb.append(wl)
                    wr = wp.tile([P, HID], f16, name=f"wr_{k}_{kc}", tag=f"wr{kc}")
                    nc.sync.dma_start(
                        out=wr[:], in_=wrt_t.ap()[k, kc * P:(kc + 1) * P, :])
                    wr_sb.append(wr)

                for gg in range(NG):
                    nchunks = int(nch1[k, gg])
                    cbase = int(base1[k, gg])
                    # --- aggregation: mean_k^T for this node group ---
                    # issue all chunk gathers/A-loads up-front (bufs give the
                    # scheduler prefetch depth), run the matmuls cc-outer so
                    # each 128-ch slice is evicted as soon as it completes and
                    # the dense stage can start with zero PE bubble.
                    gts, avs = [], []
                    for ci in range(nchunks):
                        j = cbase + ci
                        gth = iop.tile([P, 512], f16, name=f"g1_{k}_{gg}_{ci}",
                                       tag="gth", bufs=8)
                        nc.gpsimd.indirect_dma_start(
                            out=gth[:], out_offset=None,
                            in_=xtab_t.ap(),
                            in_offset=bass.IndirectOffsetOnAxis(
                                ap=idx1_sb[:, j:j + 1], axis=0))
                        av = iop.tile([P, 512], f16, name=f"a1_{k}_{gg}_{ci}",
                                      tag="av", bufs=8)
                        nc.sync.dma_start(out=av[:], in_=a1v_t.ap()[j])
                        gts.append(gth)
                        avs.append(av)
                    mean_ps = pp.tile([P, 2048], f32, space="PSUM",
                                      name=f"agg_{k}_{gg}", tag="big")
                    mean_sb = []
                    for cc in range(4):
                        for ci in range(nchunks):
                            nc.tensor.matmul(
                                out=mean_ps[:, cc * 512:(cc + 1) * 512],
                                lhsT=gts[ci][:, cc * P:(cc + 1) * P],
                                rhs=avs[ci][:],
                                start=(ci == 0), stop=(ci == nchunks - 1))
                        m = actp.tile([P, 512], f16, name=f"mean_{k}_{gg}_{cc}",
                                      tag=f"mean{cc}")
                        nc.vector.tensor_copy(
                            out=m[:], in_=mean_ps[:, cc * 512:(cc + 1) * 512])
                        mean_sb.append(m)

                    # --- dense: h = relu(Wl@mean + Wr@x + b), contraction
                    # (kc) outer so the first matmuls only need mean_sb[0] ---
                    h_ps = pp.tile([P, 2048], f32, space="PSUM",
                                   name=f"h_{k}_{gg}", tag="big")
                    for kc in range(4):
                        for mc in range(4):
                            nc.tensor.matmul(
                                out=h_ps[:, mc * 512:(mc + 1) * 512],
                                lhsT=wl_sb[kc][:, mc * P:(mc + 1) * P],
                                rhs=mean_sb[kc][:], start=(kc == 0), stop=False)
                    for kc in range(4):
                        for mc in range(4):
                            nc.tensor.matmul(
                                out=h_ps[:, mc * 512:(mc + 1) * 512],
                                lhsT=wr_sb[kc][:, mc * P:(mc + 1) * P],
                                rhs=xt_sb[gg][kc][:], start=False, stop=(kc == 3))
                    for mc in range(4):
                        r = actp.tile([P, 512], f16, name=f"relu_{k}_{gg}_{mc}",
                                      tag=f"relu{mc}")
                        nc.vector.tensor_scalar(
                            out=r[:], in0=h_ps[:, mc * 512:(mc + 1) * 512],
                            scalar1=blb_sb[:, k * 4 + mc:k * 4 + mc + 1],
                            scalar2=0.0,
                            op0=mybir.AluOpType.add, op1=mybir.AluOpType.max)
                        nc.sync.dma_start(
                            out=hrelu.ap()[k * 512 + mc * P:k * 512 + (mc + 1) * P,
                                           gg * 512:(gg + 1) * 512],
                            in_=r[:])

            # ====== Phase 2a: aggregated-side projections -> transpose -> ag_in
            for gg in range(NG):
                hr_sb = []
                for r in range(20):
                    tl = actp.tile([P, 512], f16, name=f"hrA_{gg}_{r}",
                                   tag=f"hr{r}", bufs=1)
                    dma_eng = nc.sync if r % 2 == 0 else nc.scalar
                    dma_eng.dma_start(
                        out=tl[:],
                        in_=hrelu.ap()[r * P:(r + 1) * P, gg * 512:(gg + 1) * 512])
                    hr_sb.append(tl)
                tab_sb = []
                for mc in range(4):
                    p_ps = pp.tile([P, 2048], f32, space="PSUM",
                                   name=f"proj_{gg}_{mc}", tag="big")
                    o = p_ps[:, 0:512]
                    for r in range(20):
                        nc.tensor.matmul(
                            out=o, lhsT=wall_sb[r][:, mc * P:(mc + 1) * P],
                            rhs=hr_sb[r][:], start=(r == 0), stop=(r == 19))
                    tab = actp.tile([P, 512], f16, name=f"tab_{gg}_{mc}",
                                    tag=f"tab{mc}")
                    nc.vector.tensor_scalar(
                        out=tab[:], in0=o, scalar1=twb_sb[:, mc:mc + 1],
                        scalar2=None, op0=mybir.AluOpType.add)
                    tab_sb.append(tab)
                # batched transpose: all 16 [128,128] transposes into one PSUM
                # tile, laid out so each node-subchunk is a contiguous 512 run
                tr_ps = pp.tile([P, 2048], f16, space="PSUM",
                                name=f"tr_{gg}", tag="big")
                for ns in range(4):
                    for mc in range(4):
                        nc.tensor.transpose(
                            out=tr_ps[:, (ns * 4 + mc) * P:(ns * 4 + mc + 1) * P],
                            in_=tab_sb[mc][:, ns * P:(ns + 1) * P],
                            identity=ident[:])
                for ns in range(4):
                    agin = actp.tile([P, 512], f16, name=f"agin_{gg}_{ns}",
                                     tag=f"agin{ns}")
                    nc.vector.tensor_copy(
                        out=agin[:], in_=tr_ps[:, ns * 512:(ns + 1) * 512])
                    nc.sync.dma_start(
                        out=ag_in.ap()[gg * 512 + ns * P:gg * 512 + (ns + 1) * P, :],
                        in_=agin[:])

            # =========== Phase 3: AllGather the projected table ==============
            nc.gpsimd.collective_compute(
                "AllGather", mybir.AluOpType.bypass,
                replica_groups=[list(range(NCORES))],
                ins=[ag_in.ap()], outs=[ag_tab.ap()])

            # ====== Phase 2b: self-side projections (overlap the collective)
            for gg in range(NG):
                hr_sb = []
                for r in range(20):
                    tl = actp.tile([P, 512], f16, name=f"hrB_{gg}_{r}",
                                   tag=f"hr{r}", bufs=1)
                    dma_eng = nc.sync if r % 2 == 0 else nc.scalar
                    dma_eng.dma_start(
                        out=tl[:],
                        in_=hrelu.ap()[r * P:(r + 1) * P, gg * 512:(gg + 1) * 512])
                    hr_sb.append(tl)
                for mc in range(4):
                    p_ps = pp.tile([P, 2048], f32, space="PSUM",
                                   name=f"self_{gg}_{mc}", tag="big")
                    o = p_ps[:, 0:512]
                    for r in range(20):
                        nc.tensor.matmul(
                            out=o, lhsT=wall_sb[r][:, (4 + mc) * P:(5 + mc) * P],
                            rhs=hr_sb[r][:], start=(r == 0), stop=(r == 19))
                    sf = resp.tile([P, 512], f16, name=f"selfr_{gg}_{mc}",
                                   tag=f"self_{gg}_{mc}")
                    nc.vector.tensor_scalar(
                        out=sf[:], in0=o, scalar1=bsb_sb[:, mc:mc + 1],
                        scalar2=None, op0=mybir.AluOpType.add)
                    self_res[gg][mc] = sf

            # =========== Phase 4: layer-2 aggregation + combine ==============
            for gg in range(NG):
                nchunks = int(nch2[gg])
                cbase = int(base2[gg])
                m2_ps = pp.tile([P, 2048], f32, space="PSUM",
                                name=f"m2_{gg}", tag="big")
                for ci in range(nchunks):
                    j = cbase + ci
                    gth = iop.tile([P, 512], f16, name=f"g2_{gg}_{ci}",
                                   tag="gth", bufs=8)
                    nc.gpsimd.indirect_dma_start(
                        out=gth[:], out_offset=None,
                        in_=ag_tab.ap(),
                        in_offset=bass.IndirectOffsetOnAxis(
                            ap=idx2_sb[:, j:j + 1], axis=0))
                    av = iop.tile([P, 512], f16, name=f"a2_{gg}_{ci}",
                                  tag="av", bufs=8)
                    nc.sync.dma_start(out=av[:], in_=a2v_t.ap()[j])
                    for cc in range(4):
                        nc.tensor.matmul(
                            out=m2_ps[:, cc * 512:(cc + 1) * 512],
                            lhsT=gth[:, cc * P:(cc + 1) * P],
                            rhs=av[:],
                            start=(ci == 0), stop=(ci == nchunks - 1))
                for mc in range(4):
                    ob = actp.tile([P, 512], f32, name=f"out_{gg}_{mc}", tag="outsb")
                    nc.vector.tensor_tensor(
                        out=ob[:], in0=m2_ps[:, mc * 512:(mc + 1) * 512],
                        in1=self_res[gg][mc][:], op=mybir.AluOpType.add)
                    nc.sync.dma_start(
                        out=out_t.ap()[mc * P:(mc + 1) * P,
                                       gg * 512:(gg + 1) * 512],
                        in_=ob[:])

    nc.compile()
    return nc


# ----------------------------------------------------------------------------
# Entry point
# ----------------------------------------------------------------------------

_CACHE = {}


def build_and_run(inputs, trace=False, trace_kwargs=None):
    from concourse import bass_utils

    meta, in_maps = _preprocess(**inputs)
    if meta not in _CACHE:
        _CACHE[meta] = _build(meta)
    nc = _CACHE[meta]
    res = bass_utils.run_bass_kernel_spmd(
        nc, in_maps, core_ids=list(range(NCORES)),
        trace=trace, **(trace_kwargs or {}))

    mu = np.empty((N, OUT), np.float32)
    lv = np.empty((N, OUT), np.float32)
    for c in range(NCORES):
        blk = res.results[c]["out"]            # [512, 2560] fp32
        mu[c * NLOC:(c + 1) * NLOC] = blk[0:OUT, :NLOC].T
        lv[c * NLOC:(c + 1) * NLOC] = blk[OUT:2 * OUT, :NLOC].T
    return (mu, lv), res


def kernel(**inputs):
    out, _ = build_and_run(inputs, trace=False)
    return out


# revision 10
# speedup vs baseline: 1.3363x; 1.0477x over previous
"""Trainium2 Bass kernel for a 2-layer relational GraphSAGE VGAE encoder.

Contract: kernel(**inputs) takes the FULL unsharded inputs (as produced by
setup_inputs()) and returns the full (mu, logvar) tuple.

Strategy (8 NeuronCores, SPMD single NEFF):
  - Nodes block-sharded: core c owns nodes [c*2500, (c+1)*2500), padded to 2560.
  - Edges partitioned by destination-node owner; segment-mean is computed
    locally as a sequence of (gather 128 src rows) @ (host-built one-hot with
    1/cnt weights) matmuls that directly emit the feature-major result.
  - Everything on device is feature-major [channels, nodes] fp16 with fp32
    PSUM accumulation. BatchNorm (eval) is folded into the layer-2 weights on
    the host; layer-2 has a single stacked projection weight [2560 -> 1024]
    covering mu_l|lv_l (aggregated side) and mu_r|lv_r (self side).
  - The aggregated-side projections are PE-transposed to node-major and
    AllGather'd (fp16) so each core can gather h-projections of any source
    node; the self-side matmuls overlap the collective.
"""
import sys

sys.path.insert(0, "/opt/trn_rl_repo")

import numpy as np

NCORES = 8
N = 20000
E = 100000
IN = 512
HID = 512
CAT = 2560
OUT = 256
BN_EPS = 1e-5

NLOC = N // NCORES          # 2500
NPAD = 2560                 # 20 * 128, 5 * 512
NG = NPAD // 512            # 5 node groups of 512 per core
NREL = 5
P = 128
TABROWS = NCORES * NPAD     # 20480


# ----------------------------------------------------------------------------
# Host-side preprocessing: sharding, edge chunking, weight folding
# ----------------------------------------------------------------------------

def _chunk_edges(key, ncells, src_vals, col, val):
    """Group edges by per-core cell, chunk each cell into 128-edge chunks.

    key: [E] int = core * ncells + cell   (cell < ncells)
    src_vals: [E] int32 gather row index for each edge
    col: [E] int in [0, 512) one-hot column (dst position within node group)
    val: [E] f32 one-hot value (1/cnt)

    Returns: nch [ncells] shared chunk counts (max over cores, >=1),
             base [ncells] chunk base offsets, Ctot,
             idxT [NCORES, 128, Ctot] int32, vals [NCORES, Ctot, 128, 512] f16
    """
    counts = np.bincount(key, minlength=NCORES * ncells).reshape(NCORES, ncells)
    nch = np.maximum((counts + P - 1) // P, 1).max(axis=0)  # [ncells]
    base = np.concatenate([[0], np.cumsum(nch)[:-1]])
    Ctot = int(nch.sum())

    order = np.argsort(key, kind="stable")
    ks = key[order]
    # position of each sorted edge within its (core, cell) run
    first_of_run = np.r_[True, ks[1:] != ks[:-1]]
    run_starts = np.flatnonzero(first_of_run)
    run_id = np.cumsum(first_of_run) - 1
    pos = np.arange(len(ks)) - run_starts[run_id]

    core_s = ks // ncells
    cell_s = ks % ncells
    chunk_s = base[cell_s] + pos // P
    row_s = pos % P

    idxT = np.zeros((NCORES, P, Ctot), np.int32)
    vals = np.zeros((NCORES, Ctot, P, 512), np.float16)
    idxT[core_s, row_s, chunk_s] = src_vals[order]
    vals[core_s, chunk_s, row_s, col[order]] = val[order]
    return nch, base, Ctot, idxT, vals


def _preprocess(x, edge_index, edge_attr, Wl5, Wr5, bl5,
                Wmu_l, Wmu_r, bmu, Wlv_l, Wlv_r, blv,
                gamma, beta, run_mean, run_var):
    x = np.asarray(x, np.float32)
    src = np.asarray(edge_index[0], np.int64)
    dst = np.asarray(edge_index[1], np.int64)
    rel = np.asarray(edge_attr, np.int64)

    # --- per-node degree counts ---
    cnt1 = np.bincount(rel * N + dst, minlength=NREL * N).reshape(NREL, N)
    inv1 = 1.0 / np.maximum(cnt1, 1.0)
    cnt2 = np.bincount(dst, minlength=N)
    inv2 = 1.0 / np.maximum(cnt2, 1.0)

    core = dst // NLOC
    loc = dst % NLOC
    g = loc // 512
    col = loc % 512

    # layer-1 cells: (rel, group); gather rows straight from x table
    key1 = (core * NREL + rel) * NG + g
    nch1, base1, C1, a1i, a1v = _chunk_edges(
        key1, NREL * NG, src.astype(np.int32), col, inv1[rel, dst])

    # layer-2 cells: (group); gather rows from the all-gathered table.
    # The table is assembled by NG per-group AllGathers, so its row layout is
    # [g][core][col]: row = g*8*512 + core*512 + col.
    src_loc = src % NLOC
    tabrow = ((src_loc // 512) * (NCORES * 512) + (src // NLOC) * 512
              + src_loc % 512).astype(np.int32)
    key2 = core * NG + g
    nch2, base2, C2, a2i, a2v = _chunk_edges(key2, NG, tabrow, col, inv2[dst])

    # --- node features ---
    xtab = x.astype(np.float16)                           # [N, 512] gather table
    xt = np.zeros((NCORES, IN, NPAD), np.float16)         # feature-major local x
    for c in range(NCORES):
        xt[c, :, :NLOC] = x[c * NLOC:(c + 1) * NLOC].T

    # --- weight folding (BN eval folded into layer-2 weights) ---
    f64 = np.float64
    s = np.asarray(gamma, f64) / np.sqrt(np.asarray(run_var, f64) + BN_EPS)
    t = np.asarray(beta, f64) - np.asarray(run_mean, f64) * s

    wlt = np.ascontiguousarray(
        np.asarray(Wl5, np.float32).transpose(0, 2, 1)).astype(np.float16)
    wrt = np.ascontiguousarray(
        np.asarray(Wr5, np.float32).transpose(0, 2, 1)).astype(np.float16)

    Wtab = np.concatenate([np.asarray(Wmu_l, f64), np.asarray(Wlv_l, f64)], 0)
    Wself = np.concatenate([np.asarray(Wmu_r, f64), np.asarray(Wlv_r, f64)], 0)
    Wall = np.concatenate([Wtab * s[None, :], Wself * s[None, :]], 0)  # [1024, 2560]
    wallt = np.ascontiguousarray(Wall.T).astype(np.float16)            # [2560, 1024]

    tW = (Wtab @ t).astype(np.float32)                                  # [512]
    bself = (Wself @ t + np.concatenate(
        [np.asarray(bmu, f64), np.asarray(blv, f64)])).astype(np.float32)

    # bias tiles, laid out [128, n] so a column is a per-partition scalar
    blb = np.ascontiguousarray(
        np.asarray(bl5, np.float32).reshape(NREL * 4, P).T)   # [128, 20]
    twb = np.ascontiguousarray(tW.reshape(4, P).T)            # [128, 4]
    bsb = np.ascontiguousarray(bself.reshape(4, P).T)         # [128, 4]

    meta = (tuple(nch1), tuple(base1), C1, tuple(nch2), tuple(base2), C2)
    in_maps = []
    for c in range(NCORES):
        in_maps.append({
            "xtab": xtab, "xt": xt[c],
            "a1i": a1i[c], "a1v": a1v[c],
            "a2i": a2i[c], "a2v": a2v[c],
            "wlt": wlt, "wrt": wrt, "wallt": wallt,
            "blb": blb, "twb": twb, "bsb": bsb,
        })
    return meta, in_maps


# ----------------------------------------------------------------------------
# Device kernel
# ----------------------------------------------------------------------------

def _build(meta):
    import concourse.bacc as bacc
    import concourse.bass as bass
    import concourse.tile as tile
    import concourse.mybir as mybir
    from concourse.masks import make_identity

    nch1, base1, C1, nch2, base2, C2 = meta
    nch1 = np.asarray(nch1).reshape(NREL, NG)
    base1 = np.asarray(base1).reshape(NREL, NG)
    nch2 = np.asarray(nch2)
    base2 = np.asarray(base2)

    f16, f32, i32 = mybir.dt.float16, mybir.dt.float32, mybir.dt.int32

    nc = bacc.Bacc("TRN2", target_bir_lowering=False, debug=False,
                   num_devices=NCORES)

    xtab_t = nc.dram_tensor("xtab", [N, IN], f16, kind="ExternalInput")
    xt_t = nc.dram_tensor("xt", [IN, NPAD], f16, kind="ExternalInput")
    a1i_t = nc.dram_tensor("a1i", [P, C1], i32, kind="ExternalInput")
    a1v_t = nc.dram_tensor("a1v", [C1, P, 512], f16, kind="ExternalInput")
    a2i_t = nc.dram_tensor("a2i", [P, C2], i32, kind="ExternalInput")
    a2v_t = nc.dram_tensor("a2v", [C2, P, 512], f16, kind="ExternalInput")
    wlt_t = nc.dram_tensor("wlt", [NREL, IN, HID], f16, kind="ExternalInput")
    wrt_t = nc.dram_tensor("wrt", [NREL, IN, HID], f16, kind="ExternalInput")
    wallt_t = nc.dram_tensor("wallt", [CAT, 1024], f16, kind="ExternalInput")
    blb_t = nc.dram_tensor("blb", [P, NREL * 4], f32, kind="ExternalInput")
    twb_t = nc.dram_tensor("twb", [P, 4], f32, kind="ExternalInput")
    bsb_t = nc.dram_tensor("bsb", [P, 4], f32, kind="ExternalInput")
    out_t = nc.dram_tensor("out", [512, NPAD], f32, kind="ExternalOutput")

    hrelu = nc.dram_tensor("hrelu", [CAT, NPAD], f16, kind="Internal")
    ag_in = nc.dram_tensor("ag_in", [NPAD, 512], f16, kind="Internal")
    ag_tab = nc.dram_tensor("ag_tab", [TABROWS, 512], f16,
                            kind="Internal", addr_space="Shared")

    with tile.TileContext(nc) as tc:
        with (
            tc.tile_pool(name="constp", bufs=1) as constp,
            tc.tile_pool(name="resp", bufs=1) as resp,
            tc.tile_pool(name="wp", bufs=2) as wp,
            tc.tile_pool(name="iop", bufs=3) as iop,
            tc.tile_pool(name="actp", bufs=2) as actp,
            tc.tile_pool(name="psum", bufs=2, space="PSUM") as pp,
        ):
            # ---- constants / resident tiles ----
            # (bulk resident preloads go on the scalar HWDGE ring so the sync
            # ring serves the steady-state A-value/weight stream immediately)
            ident = constp.tile([P, P], f16, name="ident", tag="ident")
            make_identity(nc, ident[:])
            blb_sb = constp.tile([P, NREL * 4], f32, name="blb_sb", tag="blb")
            nc.scalar.dma_start(out=blb_sb[:], in_=blb_t.ap())
            twb_sb = constp.tile([P, 4], f32, name="twb_sb", tag="twb")
            nc.scalar.dma_start(out=twb_sb[:], in_=twb_t.ap())
            bsb_sb = constp.tile([P, 4], f32, name="bsb_sb", tag="bsb")
            nc.scalar.dma_start(out=bsb_sb[:], in_=bsb_t.ap())

            idx1_sb = resp.tile([P, C1], i32, name="idx1_sb", tag="idx1")
            nc.sync.dma_start(out=idx1_sb[:], in_=a1i_t.ap())
            idx2_sb = resp.tile([P, C2], i32, name="idx2_sb", tag="idx2")
            nc.sync.dma_start(out=idx2_sb[:], in_=a2i_t.ap())

            # local x, feature-major, resident: 20 tiles [128, 512]
            xt_sb = []
            for gg in range(NG):
                row = []
                for kc in range(4):
                    tl = resp.tile([P, 512], f16, name=f"xt_{gg}_{kc}",
                                   tag=f"xt_{gg}_{kc}")
                    nc.scalar.dma_start(
                        out=tl[:],
                        in_=xt_t.ap()[kc * P:(kc + 1) * P,
                                      gg * 512:(gg + 1) * 512])
                    row.append(tl)
                xt_sb.append(row)

            # stacked projection weightT, resident: 20 tiles [128, 1024]
            wall_sb = []
            for r in range(20):
                tl = resp.tile([P, 1024], f16, name=f"wall_{r}", tag=f"wall_{r}")
                nc.scalar.dma_start(out=tl[:], in_=wallt_t.ap()[r * P:(r + 1) * P, :])
                wall_sb.append(tl)

            # self-side projections, kept resident until the final combine
            self_res = [[None] * 4 for _ in range(NG)]

            # =========== Phase 1: layer-1 relational SAGE -> hrelu ==========
            for k in range(NREL):
                wl_sb, wr_sb = [], []
                for kc in range(4):
                    wl = wp.tile([P, HID], f16, name=f"wl_{k}_{kc}", tag=f"wl{kc}")
                    nc.sync.dma_start(
                        out=wl[:], in_=wlt_t.ap()[k, kc * P:(kc + 1) * P, :])
                    wl_sb.append(wl)
                    wr = wp.tile([P, HID], f16, name=f"wr_{k}_{kc}", tag=f"wr{kc}")
                    nc.sync.dma_start(
                        out=wr[:], in_=wrt_t.ap()[k, kc * P:(kc + 1) * P, :])
                    wr_sb.append(wr)

                for gg in range(NG):
                    nchunks = int(nch1[k, gg])
                    cbase = int(base1[k, gg])
                    # --- aggregation: mean_k^T for this node group ---
                    # issue all chunk gathers/A-loads up-front (bufs give the
                    # scheduler prefetch depth), run the matmuls cc-outer so
                    # each 128-ch slice is evicted as soon as it completes and
                    # the dense stage can start with zero PE bubble.
                    gts, avs = [], []
                    for ci in range(nchunks):
                        j = cbase + ci
                        gth = iop.tile([P, 512], f16, name=f"g1_{k}_{gg}_{ci}",
                                       tag="gth", bufs=8)
                        nc.gpsimd.indirect_dma_start(
                            out=gth[:], out_offset=None,
                            in_=xtab_t.ap(),
                            in_offset=bass.IndirectOffsetOnAxis(
                                ap=idx1_sb[:, j:j + 1], axis=0))
                        av = iop.tile([P, 512], f16, name=f"a1_{k}_{gg}_{ci}",
                                      tag="av", bufs=8)
                        nc.sync.dma_start(out=av[:], in_=a1v_t.ap()[j])
                        gts.append(gth)
                        avs.append(av)
                    mean_ps = pp.tile([P, 2048], f32, space="PSUM",
                                      name=f"agg_{k}_{gg}", tag="big")
                    mean_sb = []
                    for cc in range(4):
                        for ci in range(nchunks):
                            nc.tensor.matmul(
                                out=mean_ps[:, cc * 512:(cc + 1) * 512],
                                lhsT=gts[ci][:, cc * P:(cc + 1) * P],
                                rhs=avs[ci][:],
                                start=(ci == 0), stop=(ci == nchunks - 1))
                        m = actp.tile([P, 512], f16, name=f"mean_{k}_{gg}_{cc}",
                                      tag=f"mean{cc}")
                        nc.vector.tensor_copy(
                            out=m[:], in_=mean_ps[:, cc * 512:(cc + 1) * 512])
                        mean_sb.append(m)

                    # --- dense: h = relu(Wl@mean + Wr@x + b), contraction
                    # (kc) outer so the first matmuls only need mean_sb[0] ---
                    h_ps = pp.tile([P, 2048], f32, space="PSUM",
                                   name=f"h_{k}_{gg}", tag="big")
                    for kc in range(4):
                        for mc in range(4):
                            nc.tensor.matmul(
                                out=h_ps[:, mc * 512:(mc + 1) * 512],
                                lhsT=wl_sb[kc][:, mc * P:(mc + 1) * P],
                                rhs=mean_sb[kc][:], start=(kc == 0), stop=False)
                    for kc in range(4):
                        for mc in range(4):
                            nc.tensor.matmul(
                                out=h_ps[:, mc * 512:(mc + 1) * 512],
                                lhsT=wr_sb[kc][:, mc * P:(mc + 1) * P],
                                rhs=xt_sb[gg][kc][:], start=False, stop=(kc == 3))
                    for mc in range(4):
                        r = actp.tile([P, 512], f16, name=f"relu_{k}_{gg}_{mc}",
                                      tag=f"relu{mc}")
                        nc.vector.tensor_scalar(
                            out=r[:], in0=h_ps[:, mc * 512:(mc + 1) * 512],
                            scalar1=blb_sb[:, k * 4 + mc:k * 4 + mc + 1],
                            scalar2=0.0,
                            op0=mybir.AluOpType.add, op1=mybir.AluOpType.max)
                        nc.scalar.dma_start(
                            out=hrelu.ap()[k * 512 + mc * P:k * 512 + (mc + 1) * P,
                                           gg * 512:(gg + 1) * 512],
                            in_=r[:])

            # ====== Phase 2a: aggregated-side projections -> transpose -> ag_in
            for gg in range(NG):
                hr_sb = []
                for r in range(20):
                    tl = actp.tile([P, 512], f16, name=f"hrA_{gg}_{r}",
                                   tag=f"hr{r}", bufs=1)
                    dma_eng = nc.sync if r % 2 == 0 else nc.scalar
                    dma_eng.dma_start(
                        out=tl[:],
                        in_=hrelu.ap()[r * P:(r + 1) * P, gg * 512:(gg + 1) * 512])
                    hr_sb.append(tl)
                tab_sb = []
                for mc in range(4):
                    p_ps = pp.tile([P, 2048], f32, space="PSUM",
                                   name=f"proj_{gg}_{mc}", tag="big")
                    o = p_ps[:, 0:512]
                    for r in range(20):
                        nc.tensor.matmul(
                            out=o, lhsT=wall_sb[r][:, mc * P:(mc + 1) * P],
                            rhs=hr_sb[r][:], start=(r == 0), stop=(r == 19))
                    tab = actp.tile([P, 512], f16, name=f"tab_{gg}_{mc}",
                                    tag=f"tab{mc}")
                    nc.vector.tensor_scalar(
                        out=tab[:], in0=o, scalar1=twb_sb[:, mc:mc + 1],
                        scalar2=None, op0=mybir.AluOpType.add)
                    tab_sb.append(tab)
                # batched transpose: all 16 [128,128] transposes into one PSUM
                # tile, laid out so each node-subchunk is a contiguous 512 run
                tr_ps = pp.tile([P, 2048], f16, space="PSUM",
                                name=f"tr_{gg}", tag="big")
                for ns in range(4):
                    for mc in range(4):
                        nc.tensor.transpose(
                            out=tr_ps[:, (ns * 4 + mc) * P:(ns * 4 + mc + 1) * P],
                            in_=tab_sb[mc][:, ns * P:(ns + 1) * P],
                            identity=ident[:])
                for ns in range(4):
                    agin = actp.tile([P, 512], f16, name=f"agin_{gg}_{ns}",
                                     tag=f"agin{ns}")
                    nc.vector.tensor_copy(
                        out=agin[:], in_=tr_ps[:, ns * 512:(ns + 1) * 512])
                    nc.sync.dma_start(
                        out=ag_in.ap()[gg * 512 + ns * P:gg * 512 + (ns + 1) * P, :],
                        in_=agin[:])
                # fire this group's AllGather immediately so the collectives
                # pipeline behind the remaining groups' compute
                nc.gpsimd.collective_compute(
                    "AllGather", mybir.AluOpType.bypass,
                    replica_groups=[list(range(NCORES))],
                    ins=[ag_in.ap()[gg * 512:(gg + 1) * 512, :]],
                    outs=[ag_tab.ap()[gg * NCORES * 512:(gg + 1) * NCORES * 512, :]])

            # ====== Phase 2b: self-side projections (overlap the collective)
            for gg in range(NG):
                hr_sb = []
                for r in range(20):
                    tl = actp.tile([P, 512], f16, name=f"hrB_{gg}_{r}",
                                   tag=f"hr{r}", bufs=1)
                    dma_eng = nc.sync if r % 2 == 0 else nc.scalar
                    dma_eng.dma_start(
                        out=tl[:],
                        in_=hrelu.ap()[r * P:(r + 1) * P, gg * 512:(gg + 1) * 512])
                    hr_sb.append(tl)
                for mc in range(4):
                    p_ps = pp.tile([P, 2048], f32, space="PSUM",
                                   name=f"self_{gg}_{mc}", tag="big")
                    o = p_ps[:, 0:512]
                    for r in range(20):
                        nc.tensor.matmul(
                            out=o, lhsT=wall_sb[r][:, (4 + mc) * P:(5 + mc) * P],
                            rhs=hr_sb[r][:], start=(r == 0), stop=(r == 19))
                    sf = resp.tile([P, 512], f16, name=f"selfr_{gg}_{mc}",
                                   tag=f"self_{gg}_{mc}")
                    nc.vector.tensor_scalar(
                        out=sf[:], in0=o, scalar1=bsb_sb[:, mc:mc + 1],
                        scalar2=None, op0=mybir.AluOpType.add)
                    self_res[gg][mc] = sf

            # =========== Phase 4: layer-2 aggregation + combine ==============
            for gg in range(NG):
                nchunks = int(nch2[gg])
                cbase = int(base2[gg])
                m2_ps = pp.tile([P, 2048], f32, space="PSUM",
                                name=f"m2_{gg}", tag="big")
                for ci in range(nchunks):
                    j = cbase + ci
                    gth = iop.tile([P, 512], f16, name=f"g2_{gg}_{ci}",
                                   tag="gth", bufs=8)
                    nc.gpsimd.indirect_dma_start(
                        out=gth[:], out_offset=None,
                        in_=ag_tab.ap(),
                        in_offset=bass.IndirectOffsetOnAxis(
                            ap=idx2_sb[:, j:j + 1], axis=0))
                    av = iop.tile([P, 512], f16, name=f"a2_{gg}_{ci}",
                                  tag="av", bufs=8)
                    nc.sync.dma_start(out=av[:], in_=a2v_t.ap()[j])
                    for cc in range(4):
                        nc.tensor.matmul(
                            out=m2_ps[:, cc * 512:(cc + 1) * 512],
                            lhsT=gth[:, cc * P:(cc + 1) * P],
                            rhs=av[:],
                            start=(ci == 0), stop=(ci == nchunks - 1))
                for mc in range(4):
                    ob = actp.tile([P, 512], f32, name=f"out_{gg}_{mc}", tag="outsb")
                    nc.vector.tensor_tensor(
                        out=ob[:], in0=m2_ps[:, mc * 512:(mc + 1) * 512],
                        in1=self_res[gg][mc][:], op=mybir.AluOpType.add)
                    nc.scalar.dma_start(
                        out=out_t.ap()[mc * P:(mc + 1) * P,
                                       gg * 512:(gg + 1) * 512],
                        in_=ob[:])

    nc.compile()
    return nc


# ----------------------------------------------------------------------------
# Entry point
# ----------------------------------------------------------------------------

_CACHE = {}


def build_and_run(inputs, trace=False, trace_kwargs=None):
    from concourse import bass_utils

    meta, in_maps = _preprocess(**inputs)
    if meta not in _CACHE:
        _CACHE[meta] = _build(meta)
    nc = _CACHE[meta]
    res = bass_utils.run_bass_kernel_spmd(
        nc, in_maps, core_ids=list(range(NCORES)),
        trace=trace, **(trace_kwargs or {}))

    mu = np.empty((N, OUT), np.float32)
    lv = np.empty((N, OUT), np.float32)
    for c in range(NCORES):
        blk = res.results[c]["out"]            # [512, 2560] fp32
        mu[c * NLOC:(c + 1) * NLOC] = blk[0:OUT, :NLOC].T
        lv[c * NLOC:(c + 1) * NLOC] = blk[OUT:2 * OUT, :NLOC].T
    return (mu, lv), res


def kernel(**inputs):
    out, _ = build_and_run(inputs, trace=False)
    return out
